# revision 16
# baseline (speedup 1.0000x reference)
"""DeepSeek-MoE block (B=2, S=2048, D=1024, 16 routed experts top-2, 2 shared)
on 8 Trainium2 NeuronCores.

Strategy:
  - Routing (scores/softmax/top-2) is tiny (~0.13 GFLOP) -> computed on host.
  - Routed experts are computed SPARSELY: only the top-2 experts per token.
    Gates are folded into the gathered token rows (g * u); biases folded in on
    the host, so the device only runs plain matmuls.
  - Expert-parallel: each core owns 2 routed experts (weights resident in
    SBUF). Experts are paired big-count-with-small-count so every core runs
    the same (T_big, T_small) tile counts with minimal padding.
  - The 2 shared experts collapse into one matrix (Ws0+Ws1)/2 -> data-parallel
    over tokens (512 tokens per core).
  - All device matmuls are fp16 x fp16 -> fp32 PSUM (~2.7e-4 rel err).
  - Host applies the final combine: u + scatter(routed) + gate-weighted biases
    + shared + shared bias, in fp32.

Device kernel (per core, SPMD - same NEFF on all 8 cores):
  xr [RT, 128, 1024] fp16: routed token tiles, packed [p, c*128+q] =
     x[tile*128+q, c*128+p] (contraction dim on partitions; 2KB/partition DMA).
  wr [2, 128, 8192] fp16: the core's two expert weights, packed [p, c*1024+o]
     = W[o, c*128+p].
  xs [4, 128, 1024] / ws [128, 8192] fp16: same packing for the shared job.
  yr [RT*128, 1024] fp16, ys [512, 1024] fp16: outputs.
Per 128-token tile: 8 accumulating matmuls (K chunks) x 2 N-halves of 512 into
2 PSUM banks, then DVE/ACT copy-cast fp32->fp16 to SBUF, DMA out via SWDGE.
Input DMAs round-robin both HWDGE rings (sync/scalar); weights load as
per-chunk 256KB tiles so the first matmuls start as soon as chunk 0 lands.
"""

import numpy as np

B, S, D = 2, 2048, 1024
N_R, N_S, TOP_K = 16, 2, 2
N_CORES = 8
EPC = N_R // N_CORES        # experts per core
P = 128                     # partitions / tile rows
NCH = D // P                # contraction chunks (8)
T = B * S                   # tokens (4096)
ST = T // N_CORES // P      # shared token tiles per core (4)

_CACHE = {}                 # (T_big, T_small) -> compiled Bacc


def _build_program(T_big, T_small):
    import concourse.bacc as bacc
    import concourse.mybir as mybir
    import concourse.tile as tile

    f16, f32 = mybir.dt.float16, mybir.dt.float32
    nc = bacc.Bacc("TRN2", target_bir_lowering=False, debug=False)
    RT = T_big + T_small

    xr_d = nc.dram_tensor("xr", [RT, P, NCH * P], f16, kind="ExternalInput")
    wr_d = nc.dram_tensor("wr", [EPC, P, NCH * D], f16, kind="ExternalInput")
    xs_d = nc.dram_tensor("xs", [ST, P, NCH * P], f16, kind="ExternalInput")
    ws_d = nc.dram_tensor("ws", [P, NCH * D], f16, kind="ExternalInput")
    yr_d = nc.dram_tensor("yr", [RT * P, D], f16, kind="ExternalOutput")
    ys_d = nc.dram_tensor("ys", [ST * P, D], f16, kind="ExternalOutput")

    with tile.TileContext(nc) as tc:
        with (
            tc.tile_pool(name="wpool", bufs=1) as wpool,
            # all x tiles resident: a tight bufs count makes a later x-DMA
            # wait on a slot-release sem, stalling the whole HWDGE ring FIFO
            tc.tile_pool(name="xpool", bufs=RT + ST) as xpool,
            tc.tile_pool(name="opool", bufs=6) as opool,
            tc.tile_pool(name="pspool", bufs=4, space="PSUM") as pspool,
        ):
            # input DMAs alternate between the two HWDGE rings
            rr = [nc.sync, nc.scalar]
            rr_i = [0]

            def in_dma(out, in_):
                rr[rr_i[0] % 2].dma_start(out=out, in_=in_)
                rr_i[0] += 1

            # w0 (needed first) as per-chunk 256KB tiles for fine-grained
            # deps; w1/ws (needed >15us in) as 2x 1MB tiles -> fewer DMA
            # round-trips and fewer slot sems to prebump in the preamble
            w0_tiles = [wpool.tile([P, D], f16, tag=f"w0_{c}", name=f"w0_{c}")
                        for c in range(NCH)]
            w1_tiles = [wpool.tile([P, NCH // 2, D], f16, tag=f"w1_{h}",
                                   name=f"w1_{h}") for h in range(2)]
            ws_tiles = [wpool.tile([P, NCH // 2, D], f16, tag=f"ws_{h}",
                                   name=f"ws_{h}") for h in range(2)]

            def wget(jid, c):  # -> AP [P, D] for contraction chunk c
                if jid == 0:
                    return w0_tiles[c]
                wt = w1_tiles if jid == 1 else ws_tiles
                return wt[c // (NCH // 2)][:, c % (NCH // 2), :]

            # (job id, input dram, out dram, #tiles, tile offset, weight src AP)
            jobs = [
                (0, xr_d, yr_d, T_big, 0, wr_d.ap()[0]),
                (1, xr_d, yr_d, T_small, T_big, wr_d.ap()[1]),
                (2, xs_d, ys_d, ST, 0, ws_d.ap()),
            ]

            # Input DMA emission order: x tiles interleaved with weight
            # chunks so no x tile queues behind the whole weight stream.
            # (x_j_t, w chunk) issue order; Tile keeps per-ring FIFO order.
            x_tiles = {}
            x_order = []  # (jid, t) in the order compute consumes them
            for jid, src_d, dst_d, ntiles, toff, _w in jobs:
                for t in range(ntiles):
                    x_order.append((jid, t, src_d, toff))

            def load_x(i):
                jid, t, src_d, toff = x_order[i]
                x = xpool.tile([P, NCH, P], f16, tag="x")
                in_dma(x[:], src_d.ap()[toff + t])
                x_tiles[(jid, t)] = x

            # x0, all w0 chunks, x1-x2, w1 halves, x3-x4, ws halves,
            # then the rest of the x tiles
            HD = (NCH // 2) * D
            load_x(0)
            for c in range(NCH):
                in_dma(w0_tiles[c][:], jobs[0][5][:, c * D : (c + 1) * D])
            load_x(1); load_x(2)
            in_dma(w1_tiles[0][:], jobs[1][5][:, 0:HD])
            load_x(3)
            in_dma(w1_tiles[1][:], jobs[1][5][:, HD : 2 * HD])
            load_x(4)
            in_dma(ws_tiles[0][:], jobs[2][5][:, 0:HD])
            in_dma(ws_tiles[1][:], jobs[2][5][:, HD : 2 * HD])
            for i in range(5, len(x_order)):
                load_x(i)

            # outputs ride SWDGE (gpsimd): its completion sems are separate
            # lanes (DMASW0-7), so compute-paced output DMAs never block the
            # 8 HWDGE lanes that pace the input stream
            out_engines = [nc.gpsimd]
            out_i = [0]
            n_tiles_total = RT + ST

            for jid, src_d, dst_d, ntiles, toff, _wsrc in jobs:
                for t in range(ntiles):
                    x = x_tiles[(jid, t)]
                    ps0 = pspool.tile([P, 512], f32, tag="ps0")
                    ps1 = pspool.tile([P, 512], f32, tag="ps1")
                    for c in range(NCH):
                        st, sp = (c == 0), (c == NCH - 1)
                        wc = wget(jid, c)
                        nc.tensor.matmul(
                            ps0[:], x[:, c, :], wc[:, 0:512], start=st, stop=sp
                        )
                        nc.tensor.matmul(
                            ps1[:], x[:, c, :], wc[:, 512:1024], start=st, stop=sp
                        )
                    o = opool.tile([P, D], f16, tag="o")
                    row = (toff + t) * P
                    eng = out_engines[out_i[0] % len(out_engines)]
                    out_i[0] += 1
                    if out_i[0] == n_tiles_total:
                        # final tile: quarter-granularity copies split across
                        # DVE and ACT (the Scalar DMA-ring duty is over), and
                        # each quarter ships the moment its copy lands
                        nc.vector.tensor_copy(o[:, 0:256], ps0[:, 0:256])
                        nc.scalar.copy(o[:, 512:768], ps1[:, 0:256])
                        nc.sync.dma_start(
                            out=dst_d.ap()[row : row + P, 0:256], in_=o[:, 0:256]
                        )
                        nc.scalar.dma_start(
                            out=dst_d.ap()[row : row + P, 512:768],
                            in_=o[:, 512:768],
                        )
                        nc.vector.tensor_copy(o[:, 256:512], ps0[:, 256:512])
                        nc.scalar.copy(o[:, 768:1024], ps1[:, 256:512])
                        nc.sync.dma_start(
                            out=dst_d.ap()[row : row + P, 256:512],
                            in_=o[:, 256:512],
                        )
                        nc.scalar.dma_start(
                            out=dst_d.ap()[row : row + P, 768:1024],
                            in_=o[:, 768:1024],
                        )
                    else:
                        # both copies on DVE: the Scalar sequencer doubles as a
                        # DMA-issue ring; a copy queued behind lane-chained DMA
                        # issues lands late and stalls the PE via PSUM reuse
                        nc.vector.tensor_copy(o[:, 0:512], ps0[:])
                        nc.vector.tensor_copy(o[:, 512:1024], ps1[:])
                        eng.dma_start(out=dst_d.ap()[row : row + P, :], in_=o[:])

    nc.compile()
    return nc


def kernel(u, centroids, expert_biases, Wr, br, Ws, bs):
    from concourse.bass_utils import run_bass_kernel_spmd

    out, _ = _run(u, centroids, expert_biases, Wr, br, Ws, bs,
                  run_bass_kernel_spmd, trace=False)
    return out


def _run(u, centroids, expert_biases, Wr, br, Ws, bs, runner, trace=False,
         **runner_kwargs):
    u = np.asarray(u, dtype=np.float32)
    uf = u.reshape(T, D)

    # ---- routing on host (matches jax: softmax with max-subtraction,
    #      top-k ties -> lowest index) ----
    scores = uf @ np.asarray(centroids, np.float32).T
    scores = scores + np.asarray(expert_biases, np.float32)[None, :]
    m = scores.max(axis=1, keepdims=True)
    e = np.exp(scores - m)
    sm = e / e.sum(axis=1, keepdims=True)
    order = np.argsort(-sm, axis=1, kind="stable")[:, :TOP_K]     # [T, 2]
    gates = np.take_along_axis(sm, order, axis=1)                 # [T, 2]

    # ---- dispatch: per-expert contiguous segments, padded to 128;
    #      big experts paired with small ones so tile counts are uniform ----
    flat_e = order.reshape(-1)                                    # [2T]
    tok = np.repeat(np.arange(T), TOP_K)
    gate_f = gates.reshape(-1).astype(np.float32)
    counts = np.bincount(flat_e, minlength=N_R)

    by_count = np.argsort(-counts, kind="stable")                 # desc
    bigs, smalls = by_count[:N_CORES], by_count[N_CORES:][::-1]   # pair i<->i
    T_big = max(int(np.ceil(counts[bigs].max() / P)), 1)
    T_small = max(int(np.ceil(counts[smalls].max() / P)), 1)
    RT = T_big + T_small

    expert_base = np.empty(N_R, np.int64)
    expert_base[bigs] = np.arange(N_CORES) * RT * P
    expert_base[smalls] = np.arange(N_CORES) * RT * P + T_big * P

    sort_o = np.argsort(flat_e, kind="stable")
    starts = np.concatenate([[0], np.cumsum(counts)[:-1]])
    ranks = np.empty(TOP_K * T, np.int64)
    ranks[sort_o] = np.arange(TOP_K * T) - np.repeat(starts, counts)
    pos = expert_base[flat_e] + ranks                             # [2T]

    gx = np.zeros((N_CORES * RT * P, D), np.float32)
    gx[pos] = uf[tok] * gate_f[:, None]
    gx16 = gx.astype(np.float16)

    def pack(x16):  # [R,D] -> [R/128, 128(p), NCH*128], [p, c*128+q]=x[q, c*128+p]
        t = x16.reshape(-1, P, NCH, P)                 # [t, q, c, p]
        return np.ascontiguousarray(t.transpose(0, 3, 2, 1)).reshape(-1, P, NCH * P)

    Ws32 = np.asarray(Ws, np.float32)
    bs32 = np.asarray(bs, np.float32)
    Ws_eff = (Ws32[0] + Ws32[1]) * 0.5
    bs_eff = (bs32[0] + bs32[1]) * 0.5

    def pack_w(w):  # [o,d] -> [128(p), NCH*1024], [p, c*1024+o] = w[o, c*128+p]
        wt = w.T.astype(np.float16).reshape(NCH, P, D)  # [c, p, o]
        return np.ascontiguousarray(wt.transpose(1, 0, 2)).reshape(P, NCH * D)

    ws_packed = pack_w(Ws_eff)
    Wr = np.asarray(Wr, np.float32)
    uf16 = uf.astype(np.float16)

    in_maps = []
    for k in range(N_CORES):
        xr = pack(gx16[k * RT * P : (k + 1) * RT * P])
        wr = np.stack([pack_w(Wr[bigs[k]]), pack_w(Wr[smalls[k]])])
        xs = pack(uf16[k * (T // N_CORES) : (k + 1) * (T // N_CORES)])
        in_maps.append({"xr": xr, "wr": wr, "xs": xs, "ws": ws_packed})

    key = (T_big, T_small)
    if key not in _CACHE:
        _CACHE[key] = _build_program(T_big, T_small)
    nc = _CACHE[key]

    res = runner(nc, in_maps, core_ids=list(range(N_CORES)), trace=trace,
                 **runner_kwargs)

    # ---- host combine ----
    Yr = np.concatenate([r["yr"] for r in res.results]).astype(np.float32)
    Ys = np.concatenate([r["ys"] for r in res.results]).astype(np.float32)
    routed = Yr[pos[0::TOP_K]] + Yr[pos[1::TOP_K]]
    br32 = np.asarray(br, np.float32)
    bias = gates[:, 0, None] * br32[order[:, 0]] + gates[:, 1, None] * br32[order[:, 1]]
    out = uf + routed + bias + Ys + bs_eff[None, :]
    return out.reshape(B, S, D).astype(np.float32), res


# revision 17
# speedup vs baseline: 1.0052x; 1.0052x over previous
"""DeepSeek-MoE block (B=2, S=2048, D=1024, 16 routed experts top-2, 2 shared)
on 8 Trainium2 NeuronCores.

Strategy:
  - Routing (scores/softmax/top-2) is tiny (~0.13 GFLOP) -> computed on host.
  - Routed experts are computed SPARSELY: only the top-2 experts per token.
    Gates are folded into the gathered token rows (g * u); biases folded in on
    the host, so the device only runs plain matmuls.
  - Expert-parallel: each core owns 2 routed experts (weights resident in
    SBUF). Experts are paired big-count-with-small-count so every core runs
    the same (T_big, T_small) tile counts with minimal padding.
  - The 2 shared experts collapse into one matrix (Ws0+Ws1)/2 -> data-parallel
    over tokens (512 tokens per core).
  - All device matmuls are fp16 x fp16 -> fp32 PSUM (~2.7e-4 rel err).
  - Host applies the final combine: u + scatter(routed) + gate-weighted biases
    + shared + shared bias, in fp32.

Device kernel (per core, SPMD - same NEFF on all 8 cores):
  xr [RT, 128, 1024] fp16: routed token tiles, packed [p, c*128+q] =
     x[tile*128+q, c*128+p] (contraction dim on partitions; 2KB/partition DMA).
  wr [2, 128, 8192] fp16: the core's two expert weights, packed [p, c*1024+o]
     = W[o, c*128+p].
  xs [4, 128, 1024] / ws [128, 8192] fp16: same packing for the shared job.
  yr [RT*128, 1024] fp16, ys [512, 1024] fp16: outputs.
Per 128-token tile: 8 accumulating matmuls (K chunks) x 2 N-halves of 512 into
2 PSUM banks, then DVE/ACT copy-cast fp32->fp16 to SBUF, DMA out via SWDGE.
Input DMAs round-robin both HWDGE rings (sync/scalar); weights load as
per-chunk 256KB tiles so the first matmuls start as soon as chunk 0 lands.
"""

import numpy as np

B, S, D = 2, 2048, 1024
N_R, N_S, TOP_K = 16, 2, 2
N_CORES = 8
EPC = N_R // N_CORES        # experts per core
P = 128                     # partitions / tile rows
NCH = D // P                # contraction chunks (8)
T = B * S                   # tokens (4096)
ST = T // N_CORES // P      # shared token tiles per core (4)

_CACHE = {}                 # (T_big, T_small) -> compiled Bacc


def _build_program(T_big, T_small):
    import concourse.bacc as bacc
    import concourse.mybir as mybir
    import concourse.tile as tile

    f16, f32 = mybir.dt.float16, mybir.dt.float32
    nc = bacc.Bacc("TRN2", target_bir_lowering=False, debug=False)
    RT = T_big + T_small

    xr_d = nc.dram_tensor("xr", [RT, P, NCH * P], f16, kind="ExternalInput")
    wr_d = nc.dram_tensor("wr", [EPC, P, NCH * D], f16, kind="ExternalInput")
    xs_d = nc.dram_tensor("xs", [ST, P, NCH * P], f16, kind="ExternalInput")
    ws_d = nc.dram_tensor("ws", [P, NCH * D], f16, kind="ExternalInput")
    yr_d = nc.dram_tensor("yr", [RT * P, D], f16, kind="ExternalOutput")
    ys_d = nc.dram_tensor("ys", [ST * P, D], f16, kind="ExternalOutput")

    with tile.TileContext(nc) as tc:
        with (
            tc.tile_pool(name="wpool", bufs=1) as wpool,
            # all x tiles resident: a tight bufs count makes a later x-DMA
            # wait on a slot-release sem, stalling the whole HWDGE ring FIFO
            tc.tile_pool(name="xpool", bufs=RT + ST) as xpool,
            tc.tile_pool(name="opool", bufs=6) as opool,
            tc.tile_pool(name="pspool", bufs=4, space="PSUM") as pspool,
        ):
            # input DMAs alternate between the two HWDGE rings
            rr = [nc.sync, nc.scalar]
            rr_i = [0]

            def in_dma(out, in_):
                rr[rr_i[0] % 2].dma_start(out=out, in_=in_)
                rr_i[0] += 1

            # per-chunk weight tiles (256KB each) for fine-grained deps
            def load_w(name, src_row):  # src_row: AP [P, NCH*D]
                tiles = []
                for c in range(NCH):
                    wt = wpool.tile([P, D], f16, tag=f"{name}_{c}")
                    tiles.append(wt)
                return tiles

            w_tiles = {0: load_w("w0", None), 1: load_w("w1", None),
                       2: load_w("ws", None)}

            # (job id, input dram, out dram, #tiles, tile offset, weight src AP)
            jobs = [
                (0, xr_d, yr_d, T_big, 0, wr_d.ap()[0]),
                (1, xr_d, yr_d, T_small, T_big, wr_d.ap()[1]),
                (2, xs_d, ys_d, ST, 0, ws_d.ap()),
            ]

            # Input DMA emission order: x tiles interleaved with weight
            # chunks so no x tile queues behind the whole weight stream.
            # (x_j_t, w chunk) issue order; Tile keeps per-ring FIFO order.
            x_tiles = {}
            x_order = []  # (jid, t) in the order compute consumes them
            for jid, src_d, dst_d, ntiles, toff, _w in jobs:
                for t in range(ntiles):
                    x_order.append((jid, t, src_d, toff))

            def load_x(i):
                jid, t, src_d, toff = x_order[i]
                x = xpool.tile([P, NCH, P], f16, tag="x")
                in_dma(x[:], src_d.ap()[toff + t])
                x_tiles[(jid, t)] = x

            # x0, all w0 chunks, x1-x2, all w1 chunks, x3-x4, ws chunks,
            # then the rest of the x tiles
            load_x(0)
            for c in range(NCH):
                in_dma(w_tiles[0][c][:], jobs[0][5][:, c * D : (c + 1) * D])
            load_x(1); load_x(2)
            for c in range(NCH):
                in_dma(w_tiles[1][c][:], jobs[1][5][:, c * D : (c + 1) * D])
            load_x(3); load_x(4)
            for c in range(NCH):
                in_dma(w_tiles[2][c][:], jobs[2][5][:, c * D : (c + 1) * D])
            for i in range(5, len(x_order)):
                load_x(i)

            # outputs ride SWDGE (gpsimd): its completion sems are separate
            # lanes (DMASW0-7), so compute-paced output DMAs never block the
            # 8 HWDGE lanes that pace the input stream
            out_engines = [nc.gpsimd]
            out_i = [0]
            n_tiles_total = RT + ST

            for jid, src_d, dst_d, ntiles, toff, _wsrc in jobs:
                wt = w_tiles[jid]
                for t in range(ntiles):
                    x = x_tiles[(jid, t)]
                    ps0 = pspool.tile([P, 512], f32, tag="ps0")
                    ps1 = pspool.tile([P, 512], f32, tag="ps1")
                    for c in range(NCH):
                        st, sp = (c == 0), (c == NCH - 1)
                        nc.tensor.matmul(
                            ps0[:], x[:, c, :], wt[c][:, 0:512], start=st, stop=sp
                        )
                        nc.tensor.matmul(
                            ps1[:], x[:, c, :], wt[c][:, 512:1024], start=st, stop=sp
                        )
                    o = opool.tile([P, D], f16, tag="o")
                    row = (toff + t) * P
                    eng = out_engines[out_i[0] % len(out_engines)]
                    out_i[0] += 1
                    if out_i[0] == n_tiles_total:
                        # final tile: copies on both engines (Scalar's DMA-ring
                        # duty is over), ship each half as soon as it lands
                        nc.vector.tensor_copy(o[:, 0:512], ps0[:])
                        nc.scalar.copy(o[:, 512:1024], ps1[:])
                        nc.sync.dma_start(
                            out=dst_d.ap()[row : row + P, 0:512], in_=o[:, 0:512]
                        )
                        nc.scalar.dma_start(
                            out=dst_d.ap()[row : row + P, 512:1024],
                            in_=o[:, 512:1024],
                        )
                    else:
                        # both copies on DVE: the Scalar sequencer doubles as a
                        # DMA-issue ring; a copy queued behind lane-chained DMA
                        # issues lands late and stalls the PE via PSUM reuse
                        nc.vector.tensor_copy(o[:, 0:512], ps0[:])
                        nc.vector.tensor_copy(o[:, 512:1024], ps1[:])
                        eng.dma_start(out=dst_d.ap()[row : row + P, :], in_=o[:])

    nc.compile()
    return nc


def kernel(u, centroids, expert_biases, Wr, br, Ws, bs):
    from concourse.bass_utils import run_bass_kernel_spmd

    out, _ = _run(u, centroids, expert_biases, Wr, br, Ws, bs,
                  run_bass_kernel_spmd, trace=False)
    return out


def _run(u, centroids, expert_biases, Wr, br, Ws, bs, runner, trace=False,
         **runner_kwargs):
    u = np.asarray(u, dtype=np.float32)
    uf = u.reshape(T, D)

    # ---- routing on host (matches jax: softmax with max-subtraction,
    #      top-k ties -> lowest index) ----
    scores = uf @ np.asarray(centroids, np.float32).T
    scores = scores + np.asarray(expert_biases, np.float32)[None, :]
    m = scores.max(axis=1, keepdims=True)
    e = np.exp(scores - m)
    sm = e / e.sum(axis=1, keepdims=True)
    order = np.argsort(-sm, axis=1, kind="stable")[:, :TOP_K]     # [T, 2]
    gates = np.take_along_axis(sm, order, axis=1)                 # [T, 2]

    # ---- dispatch: per-expert contiguous segments, padded to 128;
    #      big experts paired with small ones so tile counts are uniform ----
    flat_e = order.reshape(-1)                                    # [2T]
    tok = np.repeat(np.arange(T), TOP_K)
    gate_f = gates.reshape(-1).astype(np.float32)
    counts = np.bincount(flat_e, minlength=N_R)

    by_count = np.argsort(-counts, kind="stable")                 # desc
    bigs, smalls = by_count[:N_CORES], by_count[N_CORES:][::-1]   # pair i<->i
    T_big = max(int(np.ceil(counts[bigs].max() / P)), 1)
    T_small = max(int(np.ceil(counts[smalls].max() / P)), 1)
    RT = T_big + T_small

    expert_base = np.empty(N_R, np.int64)
    expert_base[bigs] = np.arange(N_CORES) * RT * P
    expert_base[smalls] = np.arange(N_CORES) * RT * P + T_big * P

    sort_o = np.argsort(flat_e, kind="stable")
    starts = np.concatenate([[0], np.cumsum(counts)[:-1]])
    ranks = np.empty(TOP_K * T, np.int64)
    ranks[sort_o] = np.arange(TOP_K * T) - np.repeat(starts, counts)
    pos = expert_base[flat_e] + ranks                             # [2T]

    gx = np.zeros((N_CORES * RT * P, D), np.float32)
    gx[pos] = uf[tok] * gate_f[:, None]
    gx16 = gx.astype(np.float16)

    def pack(x16):  # [R,D] -> [R/128, 128(p), NCH*128], [p, c*128+q]=x[q, c*128+p]
        t = x16.reshape(-1, P, NCH, P)                 # [t, q, c, p]
        return np.ascontiguousarray(t.transpose(0, 3, 2, 1)).reshape(-1, P, NCH * P)

    Ws32 = np.asarray(Ws, np.float32)
    bs32 = np.asarray(bs, np.float32)
    Ws_eff = (Ws32[0] + Ws32[1]) * 0.5
    bs_eff = (bs32[0] + bs32[1]) * 0.5

    def pack_w(w):  # [o,d] -> [128(p), NCH*1024], [p, c*1024+o] = w[o, c*128+p]
        wt = w.T.astype(np.float16).reshape(NCH, P, D)  # [c, p, o]
        return np.ascontiguousarray(wt.transpose(1, 0, 2)).reshape(P, NCH * D)

    ws_packed = pack_w(Ws_eff)
    Wr = np.asarray(Wr, np.float32)
    uf16 = uf.astype(np.float16)

    in_maps = []
    for k in range(N_CORES):
        xr = pack(gx16[k * RT * P : (k + 1) * RT * P])
        wr = np.stack([pack_w(Wr[bigs[k]]), pack_w(Wr[smalls[k]])])
        xs = pack(uf16[k * (T // N_CORES) : (k + 1) * (T // N_CORES)])
        in_maps.append({"xr": xr, "wr": wr, "xs": xs, "ws": ws_packed})

    key = (T_big, T_small)
    if key not in _CACHE:
        _CACHE[key] = _build_program(T_big, T_small)
    nc = _CACHE[key]

    res = runner(nc, in_maps, core_ids=list(range(N_CORES)), trace=trace,
                 **runner_kwargs)

    # ---- host combine ----
    Yr = np.concatenate([r["yr"] for r in res.results]).astype(np.float32)
    Ys = np.concatenate([r["ys"] for r in res.results]).astype(np.float32)
    routed = Yr[pos[0::TOP_K]] + Yr[pos[1::TOP_K]]
    br32 = np.asarray(br, np.float32)
    bias = gates[:, 0, None] * br32[order[:, 0]] + gates[:, 1, None] * br32[order[:, 1]]
    out = uf + routed + bias + Ys + bs_eff[None, :]
    return out.reshape(B, S, D).astype(np.float32), res
